# revision 22
# baseline (speedup 1.0000x reference)
"""Trainium2 Bass kernel for nn_MemoryDiscriminator.

Strategy (data-parallel over batch, 8 cores, 32 rows each):
  - Only outs[-1] of the reference scan is used -> no per-step outputs needed.
  - The memory-GRU trajectory hm(t) is completely input-independent, so the
    host computes it (float64) and only hm(127) / its similarity projection
    reach the device. The device scan carries the batch GRU only.
  - Phase A computes e16 = LeakyReLU(x @ W_emb.T + b_emb) once (single fused
    Prelu activation per tile), BN stats via bn_stats/bn_aggr, and streams
    e16 to DRAM. The host merges per-core stats exactly and folds the BN
    affine into the gate weights (W2 = w_ih_x^T * scale) and biases.
  - Phase B loads e16 from DRAM (no recompute), computes gi per 16-step chunk
    with full-width matmuls, and runs the scan with a 5-hop critical path:
      PE(r gates) -> sigmoid(r) -> tts(t2 = rs*pn + gi_n) -> tanh -> nz=zc*n
    where:
      * z-gate rows/biases are pre-negated so one sigmoid yields zc = 1-z.
      * t2 is computed in ONE DVE op via tensor_tensor_scan over interleaved
        pairs: d0 = [0 | pn] (PSUM, evens zeroed by a start matmul),
        d1 = [rs | gi_n] (SBUF f16, rs written strided by the sigmoid):
        state_even = 0*s + rs ; state_odd = pn*rs + gi_n = t2.
      * the state is kept as the pair (u, nz) with h = u + nz; the next
        step's matmuls consume u and nz directly (W.h = W.u + W.nz), so h
        materialization, zh and u updates run off-chain on GPSIMD.
"""

import numpy as np

B, T, IN, H, OUT, SIM = 256, 128, 128, 1024, 256, 4
NCORES, BS = 8, 32
GOFF = [0, 128, 256, 384, 512, 640]          # gate-dim offset: r0 r1 z0 z1 n0 n1
GSGN = [1.0, 1.0, -1.0, -1.0, 1.0, 1.0]      # z gates negated
EPS_BN, EPS_COS = 1e-5, 1e-8
NCH, CH, TCH = 8, 512, 16                    # chunks of 16 timesteps (512 = 16*32)

_cache = {}


def _f16(a):
    return np.ascontiguousarray(a, dtype=np.float16)


def _f32(a):
    return np.ascontiguousarray(a, dtype=np.float32)


def _ktile(a, k, n):
    # (k*128, n) -> (128, k*n) with column-block j = rows j*128..j*128+128
    return np.ascontiguousarray(a.reshape(k, 128, n).transpose(1, 0, 2).reshape(128, k * n))


def _build_phase_a(zero_bemb):
    from concourse import bacc
    import concourse.mybir as mybir
    import concourse.tile as tile

    f32, f16 = mybir.dt.float32, mybir.dt.float16
    AF = mybir.ActivationFunctionType
    OP = mybir.AluOpType

    nc = bacc.Bacc()
    xT = nc.declare_dram_parameter("xT", [128, BS * T], f16, isOutput=False)
    WembT = nc.declare_dram_parameter("WembT", [128, H], f16, isOutput=False)
    bemb = nc.declare_dram_parameter("bemb", [128, 8], f32, isOutput=False)
    e16d = nc.declare_dram_parameter("e16", [128, 8, BS * T], f16, isOutput=True)
    bnout = nc.declare_dram_parameter("bnout", [128, 8, 2], f32, isOutput=True)

    with tile.TileContext(nc) as tc:
        with tc.tile_pool(name="consts", bufs=1) as consts, \
             tc.tile_pool(name="emb", bufs=4) as embp, \
             tc.tile_pool(name="stat", bufs=2) as statp, \
             tc.tile_pool(name="ps", bufs=4, space="PSUM") as psum:
            xt = consts.tile([128, BS * T], f16)
            wt = consts.tile([128, H], f16)
            bt = consts.tile([128, 8], f32)
            nc.sync.dma_start(out=xt[:], in_=xT[:])
            nc.sync.dma_start(out=wt[:], in_=WembT[:])
            nc.sync.dma_start(out=bt[:], in_=bemb[:])
            out_sb = statp.tile([128, 8, 2], f32)
            for m in range(8):
                e_m = embp.tile([128, BS * T], f16, tag="em")
                stats = statp.tile([128, NCH, 6], f32, tag="st")
                for ch in range(NCH):
                    pe = psum.tile([128, CH], f32)
                    nc.tensor.matmul(pe[:], wt[:, m * 128:(m + 1) * 128],
                                     xt[:, ch * CH:(ch + 1) * CH], start=True, stop=True)
                    e_sl = e_m[:, ch * CH:(ch + 1) * CH]
                    nc.scalar.activation(e_sl, pe[:],
                                         AF.Prelu, bias=bt[:, m:m + 1], scale=1.0,
                                         alpha=0.2)
                    nc.vector.bn_stats(stats[:, ch, :], e_sl)
                nc.vector.bn_aggr(out_sb[:, m, :], stats[:, 0:NCH, :])
                nc.sync.dma_start(out=e16d[:, m, :], in_=e_m[:])
            nc.sync.dma_start(out=bnout[:], in_=out_sb[:])
    nc.finalize()
    return nc


def _build_phase_b(has_bhn):
    from concourse import bacc
    import concourse.mybir as mybir
    import concourse.tile as tile

    f32, f16 = mybir.dt.float32, mybir.dt.float16
    AF = mybir.ActivationFunctionType
    OP = mybir.AluOpType
    W = 32                                     # batch columns per k-block

    nc = bacc.Bacc()
    dp = nc.declare_dram_parameter
    e16d = dp("e16", [128, 8, BS * T], f16, isOutput=False)
    W2d = dp("W2", [128, 8 * 768], f16, isOutput=False)
    biasgd = dp("biasg", [128, 6], f32, isOutput=False)
    whxd = dp("whx", [128, 2 * 768], f16, isOutput=False)
    identd = dp("ident", [128, 128], f16, isOutput=False)
    WsxT = dp("WsxT", [128, 2 * SIM], f16, isOutput=False)
    qmcd = dp("qmc", [SIM, 1], f32, isOutput=False)      # qm / max(|qm|, eps)
    bsxd = dp("bsx", [SIM, 1], f32, isOutput=False)
    ones4 = dp("ones4", [SIM, 1], f32, isOutput=False)
    ones128 = dp("ones128", [1, 128], f32, isOutput=False)
    hm127d = dp("hm127", [128, 2], f32, isOutput=False)
    if has_bhn:
        bhnd = dp("bhn", [1, 256], f32, isOutput=False)  # bhh_x n-gate
        onesbd = dp("onesb", [1, BS], f32, isOutput=False)
    outT = dp("outT", [128, 2, BS], f32, isOutput=True)

    with tile.TileContext(nc) as tc:
        with tc.tile_pool(name="consts", bufs=1) as consts, \
             tc.tile_pool(name="gi", bufs=1) as gip, \
             tc.tile_pool(name="state", bufs=6) as statep, \
             tc.tile_pool(name="work", bufs=6) as work:
            def cl(t_, src):
                tt = consts.tile(list(src.shape), src.dtype, tag=t_)
                nc.sync.dma_start(out=tt[:], in_=src[:])
                return tt

            W2 = cl("W2", W2d); biasg = cl("biasg", biasgd)
            whx = cl("whx", whxd)
            idt = cl("idt", identd)
            wsx = cl("wsx", WsxT); qmc = cl("qmc", qmcd); bsx = cl("bsx", bsxd)
            o4 = cl("o4", ones4); o128 = cl("o128", ones128)
            hm127 = cl("hm127", hm127d)
            if has_bhn:
                bhnt = cl("bhnt", bhnd); onbt = cl("onbt", onesbd)

            # e16 in SBUF, loaded chunk-by-chunk (all m per chunk)
            e16 = consts.tile([128, 8, BS * T], f16, tag="e16")

            def load_chunk(ch):
                nc.sync.dma_start(out=e16[:, :, ch * CH:(ch + 1) * CH],
                                  in_=e16d[:, :, ch * CH:(ch + 1) * CH])

            # gi buffers
            gi_rz = gip.tile([128, T, 4, W], f16)             # r0 r1 z0 z1 blocks
            gin1 = gip.tile([128, T, 2, W, 2], f16)           # d1: [rs | gi_n]
            zz = consts.tile([128, 4 * W], f16, tag="zz")
            nc.gpsimd.memset(zz[:], 0.0)

            # state tiles: W.h(t-1) decomposed as hxm(t-2) + zh_neg(t-1) + nz(t-1)
            zh0 = statep.tile([128, 2 * W], f16, tag="zh")
            nz0 = statep.tile([128, 2 * W], f16, tag="nz")
            hxo0 = statep.tile([128, 2 * W], f16, tag="hxmo")
            hx0 = statep.tile([128, 2 * W], f16, tag="hxm")
            nc.gpsimd.memset(zh0[:], 0.0)
            nc.gpsimd.memset(nz0[:], 0.0)
            nc.gpsimd.memset(hxo0[:], 0.0)
            nc.gpsimd.memset(hx0[:], 0.0)
            state = {"zh": zh0, "nz": nz0, "hxm_old": hxo0, "hxm": hx0}

            load_chunk(0)

            with tc.tile_pool(name="rp", bufs=2, space="PSUM") as rpp, \
                 tc.tile_pool(name="zp", bufs=2, space="PSUM") as zpp, \
                 tc.tile_pool(name="pn", bufs=2, space="PSUM") as pnp, \
                 tc.tile_pool(name="pg", bufs=2, space="PSUM") as pgp:

                def b3_matmuls(ch, g, half4):
                    # half4 0: allocate tile + first 4 k; 1: last 4 k.
                    # 256-col pieces so a piece mid-flight delays the chain's
                    # recurrent matmuls by at most ~107 ns.
                    if half4 == 0:
                        pg = pgp.tile([128, CH], f32, tag="pg")
                        b3_pgs[g] = pg
                    pg = b3_pgs[g]
                    for k in range(4 * half4, 4 * half4 + 4):
                        nc.tensor.matmul(pg[:], W2[:, k * 768 + GOFF[g]:k * 768 + GOFF[g] + 128],
                                         e16[:, k, ch * CH:(ch + 1) * CH],
                                         start=(k == 0), stop=(k == 7))
                    return pg

                def b3_copy(ch, g, half):
                    # half 0/1: 8 timesteps each, 256 cols
                    pg = b3_pgs[g]
                    t0 = ch * TCH + half * 8
                    src = pg[:].rearrange("p (t b) -> p t b", b=BS)[:, half * 8:(half + 1) * 8, :]
                    if g < 4:
                        dst = gi_rz[:, t0:t0 + 8, g, :]
                    else:
                        dst = gin1[:, t0:t0 + 8, g - 4, :, 1]
                    nc.scalar.activation(dst, src, AF.Identity,
                                         bias=biasg[:, g:g + 1], scale=1.0)

                b3_pgs = {}

                def rec_mm(rhs_name, outs, stop_mask):
                    # one rhs tensor into each gate psum region
                    rhs = state[rhs_name]
                    for (ps_out, g), last in zip(outs, stop_mask):
                        for k in range(2):
                            nc.tensor.matmul(ps_out,
                                             whx[:, k * 768 + GOFF[g]:k * 768 + GOFF[g] + 128],
                                             rhs[:, k * W:(k + 1) * W],
                                             start=False,
                                             stop=(last and k == 1),
                                             skip_group_check=True)

                def scan_step(t, b3pre, b3ops):
                    # --- PE: next-chunk gi matmuls first (never block the chain) ---
                    for fn in b3pre:
                        fn()
                    pn = pnp.tile([128, 2, W, 2], f32, tag="pn")
                    rp = rpp.tile([128, 2, W], f32, tag="rp")
                    zp = zpp.tile([128, 2, W], f32, tag="zp")
                    nc.tensor.matmul(pn[:].rearrange("p g w o -> p (g w o)"),
                                     idt[:], zz[:], start=True, stop=False,
                                     skip_group_check=True)
                    nc.tensor.matmul(rp[:].rearrange("p g w -> p (g w)"), idt[:],
                                     gi_rz[:, t, 0:2, :].rearrange("p g w -> p (g w)"),
                                     start=True, stop=False, skip_group_check=True)
                    nc.tensor.matmul(zp[:].rearrange("p g w -> p (g w)"), idt[:],
                                     gi_rz[:, t, 2:4, :].rearrange("p g w -> p (g w)"),
                                     start=True, stop=False, skip_group_check=True)
                    outs = [(rp[:, 0, :], 0), (rp[:, 1, :], 1),
                            (zp[:, 0, :], 2), (zp[:, 1, :], 3),
                            (pn[:, 0, :, 1], 4), (pn[:, 1, :, 1], 5)]
                    # W.h(t-1) = W.hxm(t-2) + W.zh_neg(t-1) + W.nz(t-1):
                    # hxm_old has a full step of slack, zh arrives mid-step,
                    # only the nz matmuls sit on the chain (r gates stop first).
                    rec_mm("hxm_old", outs, [False] * 6)
                    rec_mm("zh", outs, [False] * 6)
                    rec_mm("nz", outs,
                           [True, True, True, True, not has_bhn, not has_bhn])
                    if has_bhn:
                        for g in range(2):
                            nc.tensor.matmul(pn[:, g, :, 1],
                                             bhnt[0:1, g * 128:(g + 1) * 128],
                                             onbt[0:1, :], start=False, stop=(g == 1),
                                             skip_group_check=True)
                    # --- ACT: sigmoids ---
                    nc.scalar.activation(gin1[:, t, :, :, 0:1],
                                         rp[:].rearrange("p g (w o) -> p g w o", o=1),
                                         AF.Sigmoid)
                    zc = work.tile([128, 2 * W], f16, tag="zc")
                    nc.scalar.activation(zc[:].rearrange("p (g w) -> p g w", w=W),
                                         zp[:], AF.Sigmoid)
                    # --- DVE: t2 via interleaved scan ---
                    t2i = work.tile([128, 4 * W], f16, tag="t2i")
                    nc.vector.tensor_tensor_scan(
                        t2i[:], pn[:].rearrange("p g w o -> p (g w o)"),
                        gin1[:, t, :, :, :].rearrange("p g w o -> p (g w o)"),
                        1.0, op0=OP.mult, op1=OP.add)
                    # --- DVE (off-chain): zh_neg = -zc*h(t-1), a = h(t-1)+zh_neg ---
                    hxm_prev = state["hxm"]
                    zh_new = statep.tile([128, 2 * W], f16, tag="zh")
                    nc.vector.scalar_tensor_tensor(zh_new[:], zc[:], -1.0, hxm_prev[:],
                                                   op0=OP.mult, op1=OP.mult)
                    a_new = work.tile([128, 2 * W], f16, tag="anew")
                    nc.vector.tensor_add(a_new[:], hxm_prev[:], zh_new[:])
                    # --- ACT: tanh ---
                    n16 = work.tile([128, 2 * W], f16, tag="n16")
                    nc.scalar.activation(n16[:],
                                         t2i[:].rearrange("p (w o) -> p w o", o=2)[:, :, 1:2]
                                         .rearrange("p w o -> p (w o)"),
                                         AF.Tanh)
                    # B3 work for the next chunk rides the queues here
                    for fn in b3ops:
                        fn()
                    # --- DVE: nz (chain tail) ---
                    nz_new = statep.tile([128, 2 * W], f16, tag="nz")
                    nc.vector.tensor_mul(nz_new[:], zc[:], n16[:])
                    # --- DVE: materialize hxm = a + nz (off-chain) ---
                    hx_new = statep.tile([128, 2 * W], f16, tag="hxm")
                    nc.vector.tensor_add(hx_new[:], a_new[:], nz_new[:])
                    state["hxm_old"] = state["hxm"]
                    state["zh"] = zh_new
                    state["nz"] = nz_new
                    state["hxm"] = hx_new

                # prolog: B3 for chunk 0 up front
                for g in range(6):
                    b3_matmuls(0, g, 0)
                    b3_matmuls(0, g, 1)
                    b3_copy(0, g, 0)
                    b3_copy(0, g, 1)

                for ch in range(NCH):
                    if ch + 1 < NCH:
                        load_chunk(ch + 1)
                    # spread next chunk's B3 work across this chunk's 16 steps
                    schedp, schedc = {}, {}
                    if ch + 1 < NCH:
                        for g in range(6):
                            def mm0(g=g, ch=ch):
                                b3_matmuls(ch + 1, g, 0)
                            def mm1(g=g, ch=ch):
                                b3_matmuls(ch + 1, g, 1)
                            def cp0(g=g, ch=ch):
                                b3_copy(ch + 1, g, 0)
                            def cp1(g=g, ch=ch):
                                b3_copy(ch + 1, g, 1)
                            s = 2 * g + 1
                            schedp.setdefault(s, []).append(mm0)
                            schedp.setdefault(s + 1, []).append(mm1)
                            schedc.setdefault(s + 1, []).append(cp0)
                            schedc.setdefault(s + 2, []).append(cp1)
                    for j in range(TCH):
                        scan_step(ch * TCH + j, schedp.get(j, []),
                                  schedc.get(j, []))

            # ---- final gate: hx = state (u+nz), hm from host ----
            hxm = state["hxm"]
            with tc.tile_pool(name="fin", bufs=1, space="PSUM") as finp:
                pq = finp.tile([SIM, BS], f32, tag="pq")
                for k in range(2):
                    nc.tensor.matmul(pq[:], wsx[:, k * SIM:(k + 1) * SIM],
                                     hxm[:, k * W:(k + 1) * W],
                                     start=(k == 0), stop=(k == 1), skip_group_check=True)
                qx = work.tile([SIM, BS], f32, tag="qx")
                nc.scalar.activation(qx[:], pq[:], AF.Identity,
                                     bias=bsx[:, 0:1], scale=1.0)
                sq = work.tile([SIM, BS], f32, tag="sq")
                nc.vector.tensor_mul(sq[:], qx[:], qx[:])
                cs0 = finp.tile([1, BS], f32, tag="cs0")
                cs1 = finp.tile([1, BS], f32, tag="cs1")
                nc.tensor.matmul(cs0[:], qmc[:], qx[:], start=True, stop=True,
                                 skip_group_check=True)
                nc.tensor.matmul(cs1[:], o4[:], sq[:], start=True, stop=True,
                                 skip_group_check=True)
                s_sb = work.tile([1, BS], f32, tag="ssb")
                nc.scalar.activation(s_sb[:], cs1[:], AF.Sqrt)
                nc.vector.tensor_scalar_max(s_sb[:], s_sb[:], EPS_COS)
                nc.vector.reciprocal(s_sb[:], s_sb[:])
                rat = work.tile([1, BS], f32, tag="rat")
                nc.vector.tensor_mul(rat[:], cs0[:], s_sb[:])
                g_sb = work.tile([1, BS], f32, tag="gsb")
                nc.scalar.activation(g_sb[:], rat[:], AF.Sigmoid)
                gbc = finp.tile([128, BS], f32, tag="gbc")
                nc.tensor.matmul(gbc[:], o128[:], g_sb[0:1, :], start=True, stop=True,
                                 skip_group_check=True)
                out_sb = work.tile([128, 2, BS], f32, tag="outsb")
                for k in range(2):
                    ddk = work.tile([128, BS], f32, tag="ddk")
                    nc.vector.tensor_scalar_sub(ddk[:], hxm[:, k * W:(k + 1) * W],
                                                hm127[:, k:k + 1])
                    ppk = work.tile([128, BS], f32, tag="ppk")
                    nc.vector.tensor_mul(ppk[:], gbc[:], ddk[:])
                    nc.vector.tensor_scalar_add(out_sb[:, k, :], ppk[:], hm127[:, k:k + 1])
                nc.sync.dma_start(out=outT[:], in_=out_sb[:])
    nc.finalize()
    return nc


def _prep(inputs):
    x = _f32(inputs["x"])
    meta = {k: _f32(inputs[k]) for k in
            ("W_emb", "b_emb", "gamma", "beta", "w_ih_x", "w_hh_x", "b_ih_x",
             "b_hh_x", "w_ih_m", "w_hh_m", "b_ih_m", "b_hh_m", "W_sx", "b_sx",
             "W_sm", "b_sm")}
    meta["mem"] = _f32(inputs["memory"])[0]
    meta["has_bhn"] = bool(np.any(meta["b_hh_x"][512:]))

    per_core = []
    for c in range(NCORES):
        xc = x[c * BS:(c + 1) * BS]                       # (32,T,IN)
        per_core.append(_f16(xc.transpose(2, 1, 0).reshape(IN, T * BS)))
    WembT = _f16(meta["W_emb"].T)
    bemb_t = _f32(meta["b_emb"].reshape(8, 128).T)
    return per_core, WembT, bemb_t, meta


def _host_hm(meta):
    """Memory-GRU trajectory (batch-independent), float64."""
    mem = meta["mem"].astype(np.float64)
    w_ih = meta["w_ih_m"].astype(np.float64)
    w_hh = meta["w_hh_m"].astype(np.float64)
    b_ih = meta["b_ih_m"].astype(np.float64)
    b_hh = meta["b_hh_m"].astype(np.float64)
    gi = mem @ w_ih.T + b_ih                     # (T, 768)
    h = np.zeros(OUT, np.float64)
    sig = lambda v: 1.0 / (1.0 + np.exp(-v))
    for t in range(T):
        gh = h @ w_hh.T + b_hh
        i_r, i_z, i_n = gi[t, :256], gi[t, 256:512], gi[t, 512:]
        h_r, h_z, h_n = gh[:256], gh[256:512], gh[512:]
        r = sig(i_r + h_r)
        z = sig(i_z + h_z)
        n = np.tanh(i_n + r * h_n)
        h = (1.0 - z) * n + z * h
    return h                                      # hm(127), (256,)


def _fold(meta, mean, var):
    """Host-side BN fold + gate-weight prep (z rows negated)."""
    scale = meta["gamma"] / np.sqrt(var + EPS_BN)
    shift = meta["beta"] - mean * scale
    sgn = np.repeat(np.array(GSGN, np.float32), 128)      # (768,)

    w_ih_x = meta["w_ih_x"]
    W2 = (w_ih_x * scale[None, :]) * sgn[:, None]          # (768, 1024)
    W2T = _f16(_ktile(W2.T, 8, 768))

    biasg = np.empty((128, 6), np.float32)
    for g in range(6):
        sl = slice(GOFF[g], GOFF[g] + 128)
        bias = meta["b_ih_x"][sl] + (meta["b_hh_x"][sl] if g < 4 else 0.0)
        biasg[:, g] = GSGN[g] * (w_ih_x[sl, :] @ shift + bias)

    whx = _f16(_ktile((meta["w_hh_x"] * sgn[:, None]).T, 2, 768))

    hm = _host_hm(meta)                                    # (256,) float64
    qm = meta["W_sm"].astype(np.float64) @ hm + meta["b_sm"]
    qmn = max(np.linalg.norm(qm), EPS_COS)
    qmc = _f32((qm / qmn).reshape(SIM, 1))
    hm127 = _f32(hm.reshape(2, 128).T)                     # (128, 2)

    shared = dict(
        W2=W2T, biasg=_f32(biasg), whx=whx,
        ident=_f16(np.eye(128)),
        WsxT=_f16(_ktile(meta["W_sx"].T, 2, SIM)),
        qmc=qmc, bsx=_f32(meta["b_sx"].reshape(SIM, 1)),
        ones4=_f32(np.ones((SIM, 1))),
        ones128=_f32(np.ones((1, 128))),
        hm127=hm127,
    )
    if meta["has_bhn"]:
        shared["bhn"] = _f32(meta["b_hh_x"][512:768].reshape(1, -1))
        shared["onesb"] = _f32(np.ones((1, BS)))
    return shared


def get_programs(zero_bemb=None, has_bhn=False):
    key = ("progs", bool(zero_bemb), bool(has_bhn))
    if key not in _cache:
        _cache[key] = (_build_phase_a(bool(zero_bemb)), _build_phase_b(bool(has_bhn)))
    return _cache[key]


def kernel(**inputs) -> np.ndarray:
    from concourse.bass_utils import run_bass_kernel_spmd

    per_core, WembT, bemb_t, meta = _prep(inputs)
    nc_a, nc_b = get_programs(not np.any(meta["b_emb"]), meta["has_bhn"])
    core_ids = list(range(NCORES))

    in_a = [{"xT": per_core[c], "WembT": WembT, "bemb": bemb_t}
            for c in range(NCORES)]
    res_a = run_bass_kernel_spmd(nc_a, in_a, core_ids=core_ids).results

    # exact merge of per-core BN stats (equal counts per core)
    ms = np.stack([r["bnout"].reshape(128, 8, 2)[:, :, 0].T.reshape(H)
                   for r in res_a])                       # (8, H) means
    vs = np.stack([r["bnout"].reshape(128, 8, 2)[:, :, 1].T.reshape(H)
                   for r in res_a])
    mean = ms.astype(np.float64).mean(0)
    var = (vs.astype(np.float64) + ms.astype(np.float64) ** 2).mean(0) - mean ** 2
    shared = _fold(meta, mean.astype(np.float32), var.astype(np.float32))

    in_b = []
    for c in range(NCORES):
        m = {"e16": res_a[c]["e16"].reshape(128, 8, BS * T)}
        m.update(shared)
        in_b.append(m)
    res_b = run_bass_kernel_spmd(nc_b, in_b, core_ids=core_ids).results

    out = np.empty((B, OUT), np.float32)
    for c in range(NCORES):
        o = res_b[c]["outT"].reshape(128, 2, BS)
        out[c * BS:(c + 1) * BS] = o.transpose(2, 1, 0).reshape(BS, OUT)
    return out
